# revision 1
# baseline (speedup 1.0000x reference)
"""JumpAttention Trainium2 kernel.

Problem: B=16, S=1024, H=2048, D=256.
  Q/K/K2/V = hs @ W*, 3 biased attentions + 2 aggregation attentions,
  triadic-weighted combine, output projection by Wo.

Strategy:
  - Data-parallel over batch: 2 batches per core on 8 cores; weights and
    pos_bias replicated.
  - bf16 matmuls with fp32 PSUM accumulation (compute-bound regime).
  - Scores are computed transposed (s^T[k, q]) so that exp(s^T) directly
    serves as the stationary operand of the P@V matmul - no P transpose.
  - Softmax has no max-subtraction (scores are O(6), exp is safe in fp32);
    the denominator comes from a ones-column appended to V (row-sums appear
    as one extra output column of the P@V matmul).
  - pos_bias folded multiplicatively: exp(s + b) = exp(s) * exp(b), with
    exp(pos_bias^T) precomputed on host in bf16.
  - 1/sqrt(D) folded into Wq on host (covers all three biased attentions);
    aggregation attentions apply it via the activation scale immediate.
  - softmax(triadic_weight) computed on host, baked in as immediates.
"""

import os
from contextlib import ExitStack

import numpy as np
import ml_dtypes

B, S, H, D = 16, 1024, 2048, 256
NCORES = 8
BPC = B // NCORES  # batches per core
P = 128
HT = H // P   # 16 h-tiles
KT = S // P   # 8 s-tiles
DT = D // P   # 2 d-tiles
NQ = 512      # moving free dim (q) chunk
QC = S // NQ  # 2 q chunks
HC = H // NQ  # 4 h chunks in final projection

LAST_RESULTS = None


def _build(nc, tc, ctx, mybir, make_identity, handles, w_tri):
    xT_h, wq_h, wk_h, wk2_h, wv_h, wo_h, ebT_h, out_h = handles
    dt = mybir.dt
    bf16 = dt.bfloat16
    f32 = dt.float32
    Exp = mybir.ActivationFunctionType.Exp
    mult = mybir.AluOpType.mult
    SCALE = float(D) ** -0.5
    w0, w1, w2 = (float(x) for x in w_tri)

    # ---- pools ----
    consts = ctx.enter_context(tc.tile_pool(name="consts", bufs=1))
    xpool = ctx.enter_context(tc.tile_pool(name="xpool", bufs=4))
    ppool2 = None
    actp = ctx.enter_context(tc.tile_pool(name="actp", bufs=1))
    ppool = ctx.enter_context(tc.tile_pool(name="ppool", bufs=3))
    tmpp = ctx.enter_context(tc.tile_pool(name="tmpp", bufs=4))
    psA = ctx.enter_context(tc.tile_pool(name="psA", bufs=3, space="PSUM"))
    psO = ctx.enter_context(tc.tile_pool(name="psO", bufs=3, space="PSUM"))
    psT = ctx.enter_context(tc.tile_pool(name="psT", bufs=2, space="PSUM"))

    # ---- resident constants ----
    wq_sb = consts.tile([P, HT, D], bf16, name="wq_sb")
    wk_sb = consts.tile([P, HT, D], bf16, name="wk_sb")
    wk2_sb = consts.tile([P, HT, D], bf16, name="wk2_sb")
    wv_sb = consts.tile([P, HT, D], bf16, name="wv_sb")
    for t_, h_ in ((wq_sb, wq_h), (wk_sb, wk_h), (wk2_sb, wk2_h), (wv_sb, wv_h)):
        nc.sync.dma_start(out=t_, in_=h_[:].rearrange("(t p) d -> p t d", p=P))
    # wo/ebT aren't needed until late phases - load them after batch-0's x
    # tiles so they sit behind them in the DMA queue FIFO.
    wo_sb = consts.tile([P, DT, H], bf16, name="wo_sb")
    ebT_sb = consts.tile([P, KT, S], bf16, name="ebT_sb")
    ident = consts.tile([P, P], bf16, name="ident")
    make_identity(nc, ident)

    for b in range(BPC):
        # ---- load x^T tiles (4 h-tiles per DMA to amortize issue cost) ----
        XG = 4  # h-tiles per load group
        xgs = []
        for g in range(HT // XG):
            xg = xpool.tile([P, XG, S], bf16, name="xg", tag="xg")
            nc.sync.dma_start(
                out=xg,
                in_=xT_h[b, g * XG * P:(g + 1) * XG * P, :].rearrange(
                    "(t p) q -> p t q", p=P
                ),
            )
            xgs.append(xg)
        xts = [xgs[h_t // XG][:, h_t % XG, :] for h_t in range(HT)]
        if b == 0:
            nc.sync.dma_start(
                out=ebT_sb, in_=ebT_h[:].rearrange("(t p) q -> p t q", p=P)
            )
            nc.sync.dma_start(
                out=wo_sb, in_=wo_h[:].rearrange("(t p) h -> p t h", p=P)
            )

        # ---- projections Q^T, K^T, K2^T : [d-part, q] ----
        qT = actp.tile([P, DT, S], bf16, name="qT", tag="qT", bufs=2)
        kT = actp.tile([P, DT, S], bf16, name="kT", tag="kT", bufs=2)
        k2T = actp.tile([P, DT, S], bf16, name="k2T", tag="k2T", bufs=2)
        for dst, w_sb in ((qT, wq_sb), (kT, wk_sb), (k2T, wk2_sb)):
            for d_t in range(DT):
                for q_c in range(QC):
                    ps = psA.tile([P, NQ], f32, name="ps_proj", tag="psA")
                    for h_t in range(HT):
                        nc.tensor.matmul(
                            ps,
                            lhsT=w_sb[:, h_t, d_t * P:(d_t + 1) * P],
                            rhs=xts[h_t][:, q_c * NQ:(q_c + 1) * NQ],
                            start=(h_t == 0),
                            stop=(h_t == HT - 1),
                        )
                    nc.vector.tensor_copy(dst[:, d_t, q_c * NQ:(q_c + 1) * NQ], ps)

        # ---- V' : [s-part, d + ones-column] ----
        vP = actp.tile([P, KT, D + 1], bf16, name="vP", tag="vP", bufs=2)
        for s_t in range(KT):
            ps = psA.tile([P, NQ], f32, name="ps_v", tag="psA")
            for h_t in range(HT):
                nc.tensor.matmul(
                    ps[:, :D],
                    lhsT=xts[h_t][:, s_t * P:(s_t + 1) * P],
                    rhs=wv_sb[:, h_t, :],
                    start=(h_t == 0),
                    stop=(h_t == HT - 1),
                )
            nc.vector.tensor_copy(vP[:, s_t, :D], ps[:, :D])
        nc.vector.memset(vP[:, :, D:D + 1], 1.0)

        # ---- K', K2' (row layout + ones col) via PE transpose of K^T/K2^T ----
        kP = actp.tile([P, KT, D + 1], bf16, name="kP", tag="kP", bufs=2)
        k2P = actp.tile([P, KT, D + 1], bf16, name="k2P", tag="k2P", bufs=2)
        for src, dst in ((kT, kP), (k2T, k2P)):
            for s_t in range(KT):
                for d_t in range(DT):
                    pt = psT.tile([P, P], bf16, name="pt_k", tag="psT")
                    nc.tensor.transpose(pt, src[:, d_t, s_t * P:(s_t + 1) * P], ident)
                    nc.vector.tensor_copy(dst[:, s_t, d_t * P:(d_t + 1) * P], pt)
            nc.vector.memset(dst[:, :, D:D + 1], 1.0)

        # ---- aggregation attentions ----
        # outT[d, row] = normalize(exp(scale * row @ col^T)) @ colvals, transposed
        def agg_branch(colT, rowT, colP, outT):
            for q_c in range(QC):
                pch = ppool.tile([P, KT, NQ], bf16, name="pch_a", tag="pT")
                for m_t in range(KT):
                    ps = psA.tile([P, NQ], f32, name="ps_as", tag="psA")
                    for d_t in range(DT):
                        nc.tensor.matmul(
                            ps,
                            lhsT=colT[:, d_t, m_t * P:(m_t + 1) * P],
                            rhs=rowT[:, d_t, q_c * NQ:(q_c + 1) * NQ],
                            start=(d_t == 0),
                            stop=(d_t == DT - 1),
                        )
                    nc.scalar.activation(pch[:, m_t, :], ps, Exp, scale=SCALE)
                for q_t in range(NQ // P):
                    po = psO.tile([P, D + 1], f32, name="po_a", tag="psO")
                    for m_t in range(KT):
                        nc.tensor.matmul(
                            po,
                            lhsT=pch[:, m_t, q_t * P:(q_t + 1) * P],
                            rhs=colP[:, m_t, :],
                            start=(m_t == 0),
                            stop=(m_t == KT - 1),
                        )
                    rec = tmpp.tile([P, 1], dt.float32, name="rec_a", tag="rec_a")
                    nc.vector.reciprocal(rec, po[:, D:D + 1])
                    sd = tmpp.tile([P, D], bf16, name="sd_a", tag="sd_a")
                    nc.vector.tensor_scalar_mul(sd, po[:, :D], rec)
                    s_t = q_c * (NQ // P) + q_t
                    for d_t in range(DT):
                        pt = psT.tile([P, P], bf16, name="pt_a", tag="psT")
                        nc.tensor.transpose(pt, sd[:, d_t * P:(d_t + 1) * P], ident)
                        nc.vector.tensor_copy(outT[:, d_t, s_t * P:(s_t + 1) * P], pt)

        kaT = actp.tile([P, DT, S], bf16, name="kaT", tag="kaT")
        agg_branch(k2T, kT, k2P, kaT)   # K attends to K2 -> K_agg
        k2aT = actp.tile([P, DT, S], bf16, name="k2aT", tag="k2aT")
        agg_branch(kT, k2T, kP, k2aT)   # K2 attends to K -> K2_agg

        # ---- three biased attentions + triadic combine ----
        combT = actp.tile([P, DT, S], bf16, name="combT", tag="combT")
        kTs = (kT, kaT, k2aT)
        for q_c in range(QC):
            pchs = []
            for i in range(3):
                pch = ppool.tile([P, KT, NQ], bf16, name="pch_b", tag="pT")
                for m_t in range(KT):
                    ps = psA.tile([P, NQ], f32, name="ps_bs", tag="psA")
                    for d_t in range(DT):
                        nc.tensor.matmul(
                            ps,
                            lhsT=kTs[i][:, d_t, m_t * P:(m_t + 1) * P],
                            rhs=qT[:, d_t, q_c * NQ:(q_c + 1) * NQ],
                            start=(d_t == 0),
                            stop=(d_t == DT - 1),
                        )
                    et = tmpp.tile([P, NQ], bf16, name="et", tag="et", bufs=3)
                    nc.scalar.activation(et, ps, Exp)
                    nc.vector.tensor_mul(
                        pch[:, m_t, :], et, ebT_sb[:, m_t, q_c * NQ:(q_c + 1) * NQ]
                    )
                pchs.append(pch)
            for q_t in range(NQ // P):
                pos = []
                for i in range(3):
                    po = psO.tile([P, D + 1], f32, name="po_b", tag="psO")
                    for m_t in range(KT):
                        nc.tensor.matmul(
                            po,
                            lhsT=pchs[i][:, m_t, q_t * P:(q_t + 1) * P],
                            rhs=vP[:, m_t, :],
                            start=(m_t == 0),
                            stop=(m_t == KT - 1),
                        )
                    pos.append(po)
                rec = tmpp.tile([P, 4], dt.float32, name="rec_b", tag="rec_b")
                for i in range(3):
                    nc.vector.reciprocal(rec[:, i:i + 1], pos[i][:, D:D + 1])
                acc = tmpp.tile([P, D], dt.float32, name="acc", tag="acc")
                nc.vector.tensor_scalar(
                    out=acc, in0=pos[0][:, :D], scalar1=rec[:, 0:1], scalar2=w0,
                    op0=mult, op1=mult,
                )
                t1 = tmpp.tile([P, D], dt.float32, name="t1", tag="t1")
                nc.vector.tensor_scalar(
                    out=t1, in0=pos[1][:, :D], scalar1=rec[:, 1:2], scalar2=w1,
                    op0=mult, op1=mult,
                )
                nc.vector.tensor_add(acc, acc, t1)
                t2 = tmpp.tile([P, D], dt.float32, name="t2", tag="t2")
                nc.vector.tensor_scalar(
                    out=t2, in0=pos[2][:, :D], scalar1=rec[:, 2:3], scalar2=w2,
                    op0=mult, op1=mult,
                )
                comb = tmpp.tile([P, D], bf16, name="comb", tag="comb")
                nc.vector.tensor_add(comb, acc, t2)
                s_t = q_c * (NQ // P) + q_t
                for d_t in range(DT):
                    pt = psT.tile([P, P], bf16, name="pt_c", tag="psT")
                    nc.tensor.transpose(pt, comb[:, d_t * P:(d_t + 1) * P], ident)
                    nc.vector.tensor_copy(combT[:, d_t, s_t * P:(s_t + 1) * P], pt)

        # ---- final projection: out[s, h] = combined @ Wo ----
        for s_t in range(KT):
            ostage = tmpp.tile([P, H], f32, name="ostage", tag="ostage", bufs=2)
            for h_c in range(HC):
                ps = psA.tile([P, NQ], f32, name="ps_o", tag="psA")
                for d_t in range(DT):
                    nc.tensor.matmul(
                        ps,
                        lhsT=combT[:, d_t, s_t * P:(s_t + 1) * P],
                        rhs=wo_sb[:, d_t, h_c * NQ:(h_c + 1) * NQ],
                        start=(d_t == 0),
                        stop=(d_t == DT - 1),
                    )
                nc.scalar.copy(ostage[:, h_c * NQ:(h_c + 1) * NQ], ps)
            nc.sync.dma_start(out=out_h[b, s_t * P:(s_t + 1) * P, :], in_=ostage)


def build_program(w_tri):
    import concourse.bacc as bacc
    import concourse.tile as tile
    from concourse import mybir
    from concourse.masks import make_identity

    nc = bacc.Bacc()
    dt = mybir.dt
    xT_h = nc.dram_tensor("xT", [BPC, H, S], dt.bfloat16, kind="ExternalInput")
    wq_h = nc.dram_tensor("wq", [H, D], dt.bfloat16, kind="ExternalInput")
    wk_h = nc.dram_tensor("wk", [H, D], dt.bfloat16, kind="ExternalInput")
    wk2_h = nc.dram_tensor("wk2", [H, D], dt.bfloat16, kind="ExternalInput")
    wv_h = nc.dram_tensor("wv", [H, D], dt.bfloat16, kind="ExternalInput")
    wo_h = nc.dram_tensor("wo", [D, H], dt.bfloat16, kind="ExternalInput")
    ebT_h = nc.dram_tensor("ebT", [S, S], dt.bfloat16, kind="ExternalInput")
    out_h = nc.dram_tensor("out", [BPC, S, H], dt.float32, kind="ExternalOutput")
    handles = (xT_h, wq_h, wk_h, wk2_h, wv_h, wo_h, ebT_h, out_h)

    with ExitStack() as ctx:
        tc = ctx.enter_context(tile.TileContext(nc))
        _build(nc, tc, ctx, mybir, make_identity, handles, w_tri)
    nc.compile()
    return nc


def prep_inputs(hidden_states, Wq, Wk, Wk2, Wv, Wo, triadic_weight, pos_bias):
    f32 = np.float32
    bf16 = ml_dtypes.bfloat16
    scale = float(D) ** -0.5

    t = np.asarray(triadic_weight, dtype=np.float64)
    e = np.exp(t - t.max())
    w_tri = (e / e.sum()).astype(f32)

    wq_np = (np.asarray(Wq, f32) * scale).astype(bf16)
    wk_np = np.asarray(Wk, f32).astype(bf16)
    wk2_np = np.asarray(Wk2, f32).astype(bf16)
    wv_np = np.asarray(Wv, f32).astype(bf16)
    wo_np = np.asarray(Wo, f32).astype(bf16)
    ebT_np = np.exp(np.asarray(pos_bias, f32).T).astype(bf16)
    hs = np.asarray(hidden_states, f32)
    xTs = [
        np.ascontiguousarray(hs[c * BPC:(c + 1) * BPC].transpose(0, 2, 1)).astype(bf16)
        for c in range(NCORES)
    ]
    in_maps = [
        {
            "xT": xTs[c],
            "wq": wq_np,
            "wk": wk_np,
            "wk2": wk2_np,
            "wv": wv_np,
            "wo": wo_np,
            "ebT": ebT_np,
        }
        for c in range(NCORES)
    ]
    return w_tri, in_maps


def kernel(hidden_states, Wq, Wk, Wk2, Wv, Wo, triadic_weight, pos_bias):
    global LAST_RESULTS
    from concourse.bass_utils import run_bass_kernel_spmd

    f32 = np.float32
    w_tri, in_maps = prep_inputs(
        hidden_states, Wq, Wk, Wk2, Wv, Wo, triadic_weight, pos_bias
    )
    nc = build_program(w_tri)

    if os.environ.get("KERNEL_BUILD_ONLY"):
        return np.zeros((B, S, H), f32)

    res = run_bass_kernel_spmd(nc, in_maps, core_ids=list(range(NCORES)))
    LAST_RESULTS = res
    if res.exec_time_ns:
        print(f"HW exec time: {res.exec_time_ns} ns")
    out = np.concatenate([r["out"] for r in res.results], axis=0)
    return np.ascontiguousarray(out.astype(f32))



# revision 3
# speedup vs baseline: 253.3711x; 253.3711x over previous
"""JumpAttention Trainium2 kernel.

Problem: B=16, S=1024, H=2048, D=256.
  Q/K/K2/V = hs @ W*, 3 biased attentions + 2 aggregation attentions,
  triadic-weighted combine, output projection by Wo.

Strategy:
  - Data-parallel over batch: 2 batches per core on 8 cores; weights and
    pos_bias replicated.
  - bf16 matmuls with fp32 PSUM accumulation (compute-bound regime).
  - Scores are computed transposed (s^T[k, q]) so that exp(s^T) directly
    serves as the stationary operand of the P@V matmul - no P transpose.
  - Softmax has no max-subtraction (scores are O(6), exp is safe in fp32);
    the denominator comes from a ones-column appended to V (row-sums appear
    as one extra output column of the P@V matmul).
  - pos_bias folded multiplicatively: exp(s + b) = exp(s) * exp(b), with
    exp(pos_bias^T) precomputed on host in bf16.
  - 1/sqrt(D) folded into Wq on host (covers all three biased attentions);
    aggregation attentions apply it via the activation scale immediate.
  - softmax(triadic_weight) computed on host, baked in as immediates.
  - All DRAM inputs pre-swizzled on host to per-partition-contiguous
    layout ([128, ...] with each partition's bytes contiguous) so every
    load is 128 large descriptors - fast HWDGE issue, full bandwidth.
  - DMA issue order: wq, x(batch 0), wk/wk2/wv, ebT, wo; batch b+1's x
    is prefetched right after batch b's projections release the slots.
  - Batch b+1's projections are interleaved with batch b's final
    projection so the PE has dense work while ACT/DVE drain PSUM.
"""

import os
from contextlib import ExitStack

import numpy as np
import ml_dtypes

B, S, H, D = 16, 1024, 2048, 256
NCORES = 8
BPC = B // NCORES  # batches per core
P = 128
HT = H // P   # 16 h-tiles
KT = S // P   # 8 s-tiles
DT = D // P   # 2 d-tiles
NQ = 512      # moving free dim (q) chunk
QC = S // NQ  # 2 q chunks
HC = H // NQ  # 4 h chunks in final projection
XG = 4        # h-tiles per x load group

LAST_RESULTS = None


def _build(nc, tc, ctx, mybir, make_identity, handles, w_tri):
    xT_h, wq_h, wk_h, wk2_h, wv_h, wo_h, ebT_h, out_h = handles
    dt = mybir.dt
    bf16 = dt.bfloat16
    f32 = dt.float32
    Exp = mybir.ActivationFunctionType.Exp
    mult = mybir.AluOpType.mult
    SCALE = float(D) ** -0.5
    w0, w1, w2 = (float(x) for x in w_tri)

    # ---- pools ----
    consts = ctx.enter_context(tc.tile_pool(name="consts", bufs=1))
    xpool = ctx.enter_context(tc.tile_pool(name="xpool", bufs=4))
    actp = ctx.enter_context(tc.tile_pool(name="actp", bufs=1))
    ppool = ctx.enter_context(tc.tile_pool(name="ppool", bufs=3))
    tmpp = ctx.enter_context(tc.tile_pool(name="tmpp", bufs=3))
    psA = ctx.enter_context(tc.tile_pool(name="psA", bufs=3, space="PSUM"))
    psO = ctx.enter_context(tc.tile_pool(name="psO", bufs=3, space="PSUM"))
    psT = ctx.enter_context(tc.tile_pool(name="psT", bufs=2, space="PSUM"))

    # ---- resident constants (DMA issue order matters: wq first, then
    # batch-0 x, then the rest - the Q projection is the first PE work) ----
    wq_sb = consts.tile([P, HT, D], bf16, name="wq_sb")
    wk_sb = consts.tile([P, HT, D], bf16, name="wk_sb")
    wk2_sb = consts.tile([P, HT, D], bf16, name="wk2_sb")
    wv_sb = consts.tile([P, HT, D], bf16, name="wv_sb")
    wo_sb = consts.tile([P, DT, H], bf16, name="wo_sb")
    ebT_sb = consts.tile([P, KT, S], bf16, name="ebT_sb")
    ident = consts.tile([P, P], bf16, name="ident")

    def load_xgs(b):
        xgs = []
        for g in range(HT // XG):
            xg = xpool.tile([P, XG, S], bf16, name="xg", tag="xg")
            nc.sync.dma_start(out=xg, in_=xT_h[b, :, g * XG:(g + 1) * XG, :])
            xgs.append(xg)
        return [xgs[h_t // XG][:, h_t % XG, :] for h_t in range(HT)]

    nc.sync.dma_start(out=wq_sb, in_=wq_h[:])
    xts0 = load_xgs(0)
    nc.sync.dma_start(out=wk_sb, in_=wk_h[:])
    nc.sync.dma_start(out=wk2_sb, in_=wk2_h[:])
    nc.sync.dma_start(out=wv_sb, in_=wv_h[:])
    nc.sync.dma_start(out=ebT_sb, in_=ebT_h[:])
    nc.sync.dma_start(out=wo_sb, in_=wo_h[:])
    make_identity(nc, ident)

    # ---- per-batch emitters ----

    def proj_chunks(xts):
        """Yield per-chain chunks; returns tiles via closure dict."""
        qT = actp.tile([P, DT, S], bf16, name="qT", tag="qT", bufs=2)
        kT = actp.tile([P, DT, S], bf16, name="kT", tag="kT", bufs=2)
        k2T = actp.tile([P, DT, S], bf16, name="k2T", tag="k2T", bufs=2)
        vP = actp.tile([P, KT, D + 1], bf16, name="vP", tag="vP", bufs=2)
        kP = actp.tile([P, KT, D + 1], bf16, name="kP", tag="kP", bufs=2)
        k2P = actp.tile([P, KT, D + 1], bf16, name="k2P", tag="k2P", bufs=2)
        tiles = (qT, kT, k2T, vP, kP, k2P)

        def gen():
            # Q^T, K^T, K2^T : [d-part, q]
            for dst, w_sb in ((qT, wq_sb), (kT, wk_sb), (k2T, wk2_sb)):
                for d_t in range(DT):
                    for q_c in range(QC):
                        ps = psA.tile([P, NQ], f32, name="ps_proj", tag="psA")
                        for h_t in range(HT):
                            nc.tensor.matmul(
                                ps,
                                lhsT=w_sb[:, h_t, d_t * P:(d_t + 1) * P],
                                rhs=xts[h_t][:, q_c * NQ:(q_c + 1) * NQ],
                                start=(h_t == 0),
                                stop=(h_t == HT - 1),
                            )
                        nc.vector.tensor_copy(
                            dst[:, d_t, q_c * NQ:(q_c + 1) * NQ], ps
                        )
                        yield
            # V' : [s-part, d + ones-column]
            for s_t in range(KT):
                ps = psA.tile([P, NQ], f32, name="ps_v", tag="psA")
                for h_t in range(HT):
                    nc.tensor.matmul(
                        ps[:, :D],
                        lhsT=xts[h_t][:, s_t * P:(s_t + 1) * P],
                        rhs=wv_sb[:, h_t, :],
                        start=(h_t == 0),
                        stop=(h_t == HT - 1),
                    )
                nc.vector.tensor_copy(vP[:, s_t, :D], ps[:, :D])
                yield
            nc.vector.memset(vP[:, :, D:D + 1], 1.0)
            # K', K2' (row layout + ones col) via PE transpose of K^T/K2^T
            for src, dst in ((kT, kP), (k2T, k2P)):
                for s_t in range(KT):
                    for d_t in range(DT):
                        pt = psT.tile([P, P], bf16, name="pt_k", tag="psT")
                        nc.tensor.transpose(
                            pt, src[:, d_t, s_t * P:(s_t + 1) * P], ident
                        )
                        nc.vector.tensor_copy(
                            dst[:, s_t, d_t * P:(d_t + 1) * P], pt
                        )
                    yield
                nc.vector.memset(dst[:, :, D:D + 1], 1.0)

        return tiles, gen()

    def emit_attn(tiles):
        qT, kT, k2T, vP, kP, k2P = tiles

        # aggregation attentions:
        # outT[d, row] = normalize(exp(scale * row @ col^T)) @ colvals, transposed
        def agg_branch(colT, rowT, colP, outT):
            for q_c in range(QC):
                pch = ppool.tile([P, KT, NQ], bf16, name="pch_a", tag="pT")
                for m_t in range(KT):
                    ps = psA.tile([P, NQ], f32, name="ps_as", tag="psA")
                    for d_t in range(DT):
                        nc.tensor.matmul(
                            ps,
                            lhsT=colT[:, d_t, m_t * P:(m_t + 1) * P],
                            rhs=rowT[:, d_t, q_c * NQ:(q_c + 1) * NQ],
                            start=(d_t == 0),
                            stop=(d_t == DT - 1),
                        )
                    nc.scalar.activation(pch[:, m_t, :], ps, Exp, scale=SCALE)
                for q_t in range(NQ // P):
                    po = psO.tile([P, D + 1], f32, name="po_a", tag="psO")
                    for m_t in range(KT):
                        nc.tensor.matmul(
                            po,
                            lhsT=pch[:, m_t, q_t * P:(q_t + 1) * P],
                            rhs=colP[:, m_t, :],
                            start=(m_t == 0),
                            stop=(m_t == KT - 1),
                        )
                    rec = tmpp.tile([P, 1], dt.float32, name="rec_a", tag="rec_a")
                    nc.vector.reciprocal(rec, po[:, D:D + 1])
                    sd = tmpp.tile([P, D], bf16, name="sd_a", tag="sd_a")
                    nc.vector.tensor_scalar_mul(sd, po[:, :D], rec)
                    s_t = q_c * (NQ // P) + q_t
                    for d_t in range(DT):
                        pt = psT.tile([P, P], bf16, name="pt_a", tag="psT")
                        nc.tensor.transpose(pt, sd[:, d_t * P:(d_t + 1) * P], ident)
                        nc.vector.tensor_copy(
                            outT[:, d_t, s_t * P:(s_t + 1) * P], pt
                        )

        kaT = actp.tile([P, DT, S], bf16, name="kaT", tag="kaT")
        agg_branch(k2T, kT, k2P, kaT)   # K attends to K2 -> K_agg
        k2aT = actp.tile([P, DT, S], bf16, name="k2aT", tag="k2aT")
        agg_branch(kT, k2T, kP, k2aT)   # K2 attends to K -> K2_agg

        # three biased attentions + triadic combine
        combT = actp.tile([P, DT, S], bf16, name="combT", tag="combT", bufs=2)
        kTs = (kT, kaT, k2aT)
        for q_c in range(QC):
            pchs = []
            for i in range(3):
                pch = ppool.tile([P, KT, NQ], bf16, name="pch_b", tag="pT")
                for m_t in range(KT):
                    ps = psA.tile([P, NQ], f32, name="ps_bs", tag="psA")
                    for d_t in range(DT):
                        nc.tensor.matmul(
                            ps,
                            lhsT=kTs[i][:, d_t, m_t * P:(m_t + 1) * P],
                            rhs=qT[:, d_t, q_c * NQ:(q_c + 1) * NQ],
                            start=(d_t == 0),
                            stop=(d_t == DT - 1),
                        )
                    et = tmpp.tile([P, NQ], bf16, name="et", tag="et", bufs=3)
                    nc.scalar.activation(et, ps, Exp)
                    nc.vector.tensor_mul(
                        pch[:, m_t, :], et, ebT_sb[:, m_t, q_c * NQ:(q_c + 1) * NQ]
                    )
                pchs.append(pch)
            for q_t in range(NQ // P):
                pos = []
                for i in range(3):
                    po = psO.tile([P, D + 1], f32, name="po_b", tag="psO")
                    for m_t in range(KT):
                        nc.tensor.matmul(
                            po,
                            lhsT=pchs[i][:, m_t, q_t * P:(q_t + 1) * P],
                            rhs=vP[:, m_t, :],
                            start=(m_t == 0),
                            stop=(m_t == KT - 1),
                        )
                    pos.append(po)
                rec = tmpp.tile([P, 4], dt.float32, name="rec_b", tag="rec_b")
                for i in range(3):
                    nc.vector.reciprocal(rec[:, i:i + 1], pos[i][:, D:D + 1])
                acc = tmpp.tile([P, D], dt.float32, name="acc", tag="acc")
                nc.vector.tensor_scalar(
                    out=acc, in0=pos[0][:, :D], scalar1=rec[:, 0:1], scalar2=w0,
                    op0=mult, op1=mult,
                )
                t1 = tmpp.tile([P, D], dt.float32, name="t1", tag="t1")
                nc.vector.tensor_scalar(
                    out=t1, in0=pos[1][:, :D], scalar1=rec[:, 1:2], scalar2=w1,
                    op0=mult, op1=mult,
                )
                nc.vector.tensor_add(acc, acc, t1)
                t2 = tmpp.tile([P, D], dt.float32, name="t2", tag="t2")
                nc.vector.tensor_scalar(
                    out=t2, in0=pos[2][:, :D], scalar1=rec[:, 2:3], scalar2=w2,
                    op0=mult, op1=mult,
                )
                comb = tmpp.tile([P, D], bf16, name="comb", tag="comb")
                nc.vector.tensor_add(comb, acc, t2)
                s_t = q_c * (NQ // P) + q_t
                for d_t in range(DT):
                    pt = psT.tile([P, P], bf16, name="pt_c", tag="psT")
                    nc.tensor.transpose(pt, comb[:, d_t * P:(d_t + 1) * P], ident)
                    nc.vector.tensor_copy(
                        combT[:, d_t, s_t * P:(s_t + 1) * P], pt
                    )
        return combT

    def final_chunks(b, combT):
        def gen():
            # out[s, h] = combined @ Wo
            for s_t in range(KT):
                ostage = tmpp.tile([P, H], f32, name="ostage", tag="ostage", bufs=2)
                for h_c in range(HC):
                    ps = psA.tile([P, NQ], f32, name="ps_o", tag="psA")
                    for d_t in range(DT):
                        nc.tensor.matmul(
                            ps,
                            lhsT=combT[:, d_t, s_t * P:(s_t + 1) * P],
                            rhs=wo_sb[:, d_t, h_c * NQ:(h_c + 1) * NQ],
                            start=(d_t == 0),
                            stop=(d_t == DT - 1),
                        )
                    # alternate evacuation engine: keeps neither ACT nor
                    # DVE the serial bottleneck of this phase
                    dst = ostage[:, h_c * NQ:(h_c + 1) * NQ]
                    if h_c % 2 == 0:
                        nc.scalar.copy(dst, ps)
                    else:
                        nc.vector.tensor_copy(dst, ps)
                # split the store so the tail transfer is short
                rows = slice(s_t * P, (s_t + 1) * P)
                nc.sync.dma_start(out=out_h[b, rows, :H // 2], in_=ostage[:, :H // 2])
                nc.sync.dma_start(out=out_h[b, rows, H // 2:], in_=ostage[:, H // 2:])
                yield

        return gen()

    # ---- schedule: proj(0), [prefetch x(1)], attn(0),
    #                interleave(final(0), proj(1)), attn(1), final(1) ----
    tiles0, pgen0 = proj_chunks(xts0)
    for _ in pgen0:
        pass
    xts1 = load_xgs(1)          # prefetch: issues wait on xg slot release
    combT0 = emit_attn(tiles0)

    tiles1, pgen1 = proj_chunks(xts1)
    fgen0 = final_chunks(0, combT0)
    # proj(1) has ~36 chunks, final(0) has 8: emit ~4 proj chunks per final
    done_p = False
    for fi, _ in enumerate(fgen0):
        for _ in range(5):
            try:
                next(pgen1)
            except StopIteration:
                done_p = True
                break
    if not done_p:
        for _ in pgen1:
            pass
    combT1 = emit_attn(tiles1)
    for _ in final_chunks(1, combT1):
        pass


def build_program(w_tri):
    import concourse.bacc as bacc
    import concourse.tile as tile
    from concourse import mybir
    from concourse.masks import make_identity

    nc = bacc.Bacc()
    dt = mybir.dt
    xT_h = nc.dram_tensor("xT", [BPC, P, HT, S], dt.bfloat16, kind="ExternalInput")
    wq_h = nc.dram_tensor("wq", [P, HT, D], dt.bfloat16, kind="ExternalInput")
    wk_h = nc.dram_tensor("wk", [P, HT, D], dt.bfloat16, kind="ExternalInput")
    wk2_h = nc.dram_tensor("wk2", [P, HT, D], dt.bfloat16, kind="ExternalInput")
    wv_h = nc.dram_tensor("wv", [P, HT, D], dt.bfloat16, kind="ExternalInput")
    wo_h = nc.dram_tensor("wo", [P, DT, H], dt.bfloat16, kind="ExternalInput")
    ebT_h = nc.dram_tensor("ebT", [P, KT, S], dt.bfloat16, kind="ExternalInput")
    out_h = nc.dram_tensor("out", [BPC, S, H], dt.float32, kind="ExternalOutput")
    handles = (xT_h, wq_h, wk_h, wk2_h, wv_h, wo_h, ebT_h, out_h)

    with ExitStack() as ctx:
        tc = ctx.enter_context(tile.TileContext(nc))
        _build(nc, tc, ctx, mybir, make_identity, handles, w_tri)
    nc.compile()
    return nc


def _swizzle_p(a, p=P):
    """[N*p, M] -> [p, N, M] with partition dim first (per-partition rows
    contiguous in DRAM)."""
    n = a.shape[0] // p
    return np.ascontiguousarray(
        a.reshape(n, p, *a.shape[1:]).transpose(1, 0, *range(2, a.ndim + 1))
    )


def prep_inputs(hidden_states, Wq, Wk, Wk2, Wv, Wo, triadic_weight, pos_bias):
    f32 = np.float32
    bf16 = ml_dtypes.bfloat16
    scale = float(D) ** -0.5

    t = np.asarray(triadic_weight, dtype=np.float64)
    e = np.exp(t - t.max())
    w_tri = (e / e.sum()).astype(f32)

    wq_np = _swizzle_p((np.asarray(Wq, f32) * scale).astype(bf16))
    wk_np = _swizzle_p(np.asarray(Wk, f32).astype(bf16))
    wk2_np = _swizzle_p(np.asarray(Wk2, f32).astype(bf16))
    wv_np = _swizzle_p(np.asarray(Wv, f32).astype(bf16))
    wo_np = _swizzle_p(np.asarray(Wo, f32).astype(bf16))
    ebT_np = _swizzle_p(np.exp(np.asarray(pos_bias, f32).T).astype(bf16))
    hs = np.asarray(hidden_states, f32)
    xTs = [
        np.stack(
            [
                _swizzle_p(
                    np.ascontiguousarray(hs[c * BPC + b].T).astype(bf16)
                )
                for b in range(BPC)
            ]
        )
        for c in range(NCORES)
    ]
    in_maps = [
        {
            "xT": xTs[c],
            "wq": wq_np,
            "wk": wk_np,
            "wk2": wk2_np,
            "wv": wv_np,
            "wo": wo_np,
            "ebT": ebT_np,
        }
        for c in range(NCORES)
    ]
    return w_tri, in_maps


def kernel(hidden_states, Wq, Wk, Wk2, Wv, Wo, triadic_weight, pos_bias):
    global LAST_RESULTS
    from concourse.bass_utils import run_bass_kernel_spmd

    f32 = np.float32
    w_tri, in_maps = prep_inputs(
        hidden_states, Wq, Wk, Wk2, Wv, Wo, triadic_weight, pos_bias
    )
    nc = build_program(w_tri)

    if os.environ.get("KERNEL_BUILD_ONLY"):
        return np.zeros((B, S, H), f32)

    res = run_bass_kernel_spmd(nc, in_maps, core_ids=list(range(NCORES)))
    LAST_RESULTS = res
    if res.exec_time_ns:
        print(f"HW exec time: {res.exec_time_ns} ns")
    out = np.concatenate([r["out"] for r in res.results], axis=0)
    return np.ascontiguousarray(out.astype(f32))
